# revision 23
# baseline (speedup 1.0000x reference)
"""Trainium2 Bass kernel for nn_MixedOp_35562329211102.

Computes FM[b,c] = expm( sum_o weights[o] * logm( W[o,c]^T x[b,c] W[o,c] ) )
for x: [256,16,64,64] SPD, W: [6,16,64,32], weights: [6] (simplex).

Algorithm (matmul/elementwise only, no eigendecomposition):
  logm via a "W-only inverse-scaling" iteration: A = Y/theta, W0 = A,
  W_{j+1} = W_j * q_j(W_j)^2 with q_j(w) = a_j + b_j w driving the
  spectrum [1.8e-4, 0.94] into the fit window.  log(Y) is a minimax
  linear combination of the intermediates {I, W_j, G_j = W_j q_j(W_j),
  Wf^2, Wf^3} (sup err ~4.9e-3).  12 matmuls (32x32) per logm.
  expm via scaling-squaring: X = M/8, degree-6 Taylor (Paterson-
  Stockmeyer), then 3 squarings.  6 matmuls per expm.

All matmuls run in fp16 (1 PE cycle/row vs 4 for fp32) with fp32 PSUM
accumulation; X accumulates in fp32.  Since nc.tensor.matmul computes
lhsT^T @ rhs, the T operand is built from the DVE block-transposed
iterate (tt = b*W^T + a*I) so the PE computes the true T*W orientation
-- the T^T*W orientation amplifies asymmetric fp16 noise by ~a per
product and loses 1.5 digits.  End-to-end rel_l2 vs the eigh reference
~2.9e-3 (tolerance 2e-2).

Engine split: PE matmul waves; DVE: W^T transposes, T builds, G evacs;
Act: W/P2/expm evacs + scaled accumulation terms; GpSimd (Pool): the
X += term adds.  Next chunk's BiMap is emitted inside the previous
chunk's expm squarings to fill PE dependency bubbles.

Sharding: data-parallel over batch B across 8 cores (32 batches/core).
"""

import numpy as np

import concourse.bass as bass
from concourse import bacc
import concourse.mybir as mybir
from concourse.bass import AP
from concourse.tile import TileContext

FP = mybir.dt.float32
HP = mybir.dt.float16
HPNP = np.float16
ACT_EVAC = True
AOP = mybir.AluOpType

THETA = 9.0
ITERS = [
    (2.21597522, -1.63352364),
    (2.26737473, -1.64671327),
    (2.13640456, -1.45488036),
    (2.20771912, -1.45344947),
    (2.50608591, -1.40547832),
]
NIT = len(ITERS)
# With tt built from the block-transposed iterate (tt = b*W^T + a*I = T^T
# as stored), the PE's lhsT^T @ rhs computes the stable T*W orientation
# directly, so no symmetrization waves are needed.
SYM_SET = ()
# minimax fit of log(lam*theta) over lam in [1.5e-4, 0.975]; 'one' includes
# log(theta) already.
COEF = {
    'one': -7.94576706,
    'W0': 3.66999144,
    'G0': -2.02105948, 'W1': 4.17710772,
    'G1': -2.04714649, 'W2': 3.95076294,
    'G2': -2.04280673, 'W3': 3.76377375,
    'G3': -1.74073605, 'W4': 2.98926341,
    'G4': -1.04982791, 'Wf': 3.51885679,
    'P2': -1.72842878, 'P3': 0.37647398,
}
EXPC = [1.0, 1.0, 0.5, 1.0 / 6, 1.0 / 24, 1.0 / 120, 1.0 / 720]

C, O, D, DIN = 16, 6, 32, 64
NCORES = 8

WT_KINDS = ['W0'] + [f'G{j}' for j in range(NIT)] \
    + [f'W{j}' for j in range(1, NIT)] + ['Wf', 'P2', 'P3']
WT_NCOL = len(WT_KINDS) * O


def host_wtab(weights: np.ndarray) -> np.ndarray:
    """[128, WT_NCOL] per-partition scalar table: w[o]/8 * coef."""
    w8 = weights.astype(np.float64) / 8.0
    cols = []
    for k in WT_KINDS:
        cols.append(w8 * COEF[k])
    row = np.concatenate(cols)
    return np.tile(row[None, :], (128, 1)).astype(np.float32)


def host_idt() -> np.ndarray:
    """[128, 32] fp16: 4 stacked 32x32 identities."""
    return np.tile(np.eye(D, dtype=HPNP), (4, 1))


def _bc(t, nblk):
    """broadcast a [128, D] tile AP over nblk column blocks -> [128, nblk, D]."""
    a = t[:, :]
    return AP(a.tensor, a.offset, [list(a.ap[0]), [0, nblk], [1, D]])


def _blk(ap, nblk):
    """view a [128, nblk*D] AP as [128, nblk, D]."""
    return ap.rearrange("p (n j) -> p n j", n=nblk)


def build_nc(b_loc=32, bchunk=8):
    nchunk = b_loc // bchunk
    nb = bchunk * D          # stage2 N per (o,c)
    ncols = 4 * bchunk * D   # X / wog tile width
    nblk = 4 * bchunk        # 32x32 col-blocks per wog tile
    vslab = O * D            # 192: one (e,b) slab in vt

    nc = bacc.Bacc("TRN2")
    x = nc.dram_tensor("x", [b_loc, C, DIN, DIN], HP, kind="ExternalInput")
    Wt = nc.dram_tensor("W", [O, C, DIN, D], HP, kind="ExternalInput")
    wtab_d = nc.dram_tensor("wtab", [128, WT_NCOL], FP, kind="ExternalInput")
    idt_d = nc.dram_tensor("idt", [128, D], HP, kind="ExternalInput")
    out = nc.dram_tensor("out", [b_loc, C, D, D], FP, kind="ExternalOutput")

    with TileContext(nc) as tc, (
        tc.tile_pool(name="consts", bufs=1)) as consts, (
        tc.tile_pool(name="xp", bufs=8)) as xp, (
        tc.tile_pool(name="vp", bufs=2)) as vp, (
        tc.tile_pool(name="wog", bufs=10)) as wogp, (
        tc.tile_pool(name="tp", bufs=6)) as tp, (
        tc.tile_pool(name="gp", bufs=6)) as gp, (
        tc.tile_pool(name="ct", bufs=7)) as ctp, (
        tc.tile_pool(name="outp", bufs=2)) as outp, (
        tc.tile_pool(name="accp", bufs=4)) as accp, (
        tc.tile_pool(name="xaccp", bufs=2)) as xaccp, (
        tc.tile_pool(name="s1ps", bufs=2, space="PSUM")) as s1psp, (
        tc.tile_pool(name="s2ps", bufs=2, space="PSUM")) as s2psp, (
        tc.tile_pool(name="wkps", bufs=2, space="PSUM")) as wkps:

        # ---- constants ----
        w1t = []
        for cp in range(C // 2):
            t = consts.tile([128, O * D], HP, tag=f"w1_{cp}")
            for e in range(2):
                dst = t[64 * e:64 * (e + 1), :].rearrange("p (o j) -> p o j", o=O)
                src = Wt[:, 2 * cp + e, :, :].rearrange("o p j -> p o j")
                nc.sync.dma_start(dst, src)
            w1t.append(t)
        wtab = consts.tile([128, WT_NCOL], FP, tag="wtab", name="wtab")
        nc.sync.dma_start(wtab[:, :], wtab_d[:, :])
        idt = consts.tile([128, D], HP, tag="idt", name="idt")
        nc.sync.dma_start(idt[:, :], idt_d[:, :])
        aid = []
        for j, (a, b) in enumerate(ITERS):
            t = consts.tile([128, D], HP, tag=f"aid{j}")
            nc.vector.tensor_scalar_mul(t[:, :], idt[:, :], float(a))
            aid.append(t)
        cid3 = consts.tile([128, D], HP, tag="cid3")
        nc.vector.tensor_scalar_mul(cid3[:, :], idt[:, :], float(EXPC[3]))

        def wap(kind, o):
            i = WT_KINDS.index(kind) * O + o
            return wtab[:, i:i + 1]

        def evac_copy(dst, src_):
            if ACT_EVAC:
                nc.scalar.copy(dst, src_)
            else:
                nc.vector.tensor_copy(dst, src_)

        def evac_mul(dst, src_, s):
            if ACT_EVAC:
                nc.scalar.mul(dst, src_, s)
            else:
                nc.vector.tensor_scalar_mul(dst, src_, s)

        def acc_x(xap, src_ap, wscal):
            """X += wscal * src via Act (scaled fp16 copy) + Pool (add),
            keeping DVE free for the transpose+tt chain."""
            t = accp.tile([128, ncols], HP, tag="acc", name="acc")
            nc.scalar.mul(t[:, :], src_ap, wscal)
            nc.gpsimd.tensor_tensor(xap, t[:, :], xap, op=AOP.add)

        def mmwave(dst, lhs, rhs, start=True, stop=True):
            for cb in range(nblk):
                for i in range(4):
                    sl = slice(i * D, (i + 1) * D)
                    cs = slice(cb * D, (cb + 1) * D)
                    nc.tensor.matmul(dst[sl, cs], lhs[sl, cs], rhs[sl, cs],
                                     start=start, stop=stop,
                                     tile_position=(i * D, i * D))

        # 512 fp32 cols per 2KB PSUM zero region -> 16 col-blocks
        RBLK = 16

        def mmwave_sym(dst, lhs, rhs):
            """dst = lhs^T rhs + rhs^T lhs, PSUM-accumulated.  One
            accumulation group per 2KB zero region: the first instruction
            in a region starts it, the last (second pass) stops it."""
            for ab, (l, r) in enumerate(((lhs, rhs), (rhs, lhs))):
                for cb in range(nblk):
                    for i in range(4):
                        sl = slice(i * D, (i + 1) * D)
                        cs = slice(cb * D, (cb + 1) * D)
                        nc.tensor.matmul(
                            dst[sl, cs], l[sl, cs], r[sl, cs],
                            start=(ab == 0 and cb % RBLK == 0),
                            stop=(ab == 1 and cb % RBLK == RBLK - 1),
                            skip_group_check=True,
                            tile_position=(i * D, i * D))

        def alloc_chunk():
            Xps = xaccp.tile([128, ncols], FP, tag="xacc", name="xacc")
            nc.vector.memset(Xps[:, :], 0.0)
            wog = [wogp.tile([128, ncols], HP, tag="wog", name="wog")
                   for _ in range(O)]
            return Xps, wog

        def emit_phase_a(ch, Xps, wog):
                # ===== phase A: BiMap =====
                # stage 1 (all q up front so PE isn't gated on Act evacs):
                # V[c] = x[b,c] @ W_all[c], evac to vt slabs (e, b, o, j)
                vt = []
                for q in range(4):
                    v = vp.tile([128, 2 * bchunk * vslab], HP, tag=f"v{q}",
                                name="v")
                    vt.append(v)
                for q in range(4):
                    for e in range(2):
                        cp = 2 * q + e
                        for bb in range(bchunk):
                            b = ch * bchunk + bb
                            xt = xp.tile([128, DIN], HP, tag="xt", name="xt")
                            xsrc = x[b, 2 * cp:2 * cp + 2].rearrange(
                                "c p j -> (c p) j")
                            nc.sync.dma_start(xt[:, :], xsrc)
                            ps1 = s1psp.tile([128, O * D], FP, tag="s1",
                                             name="s1")
                            nc.tensor.matmul(ps1[0:64, :], xt[0:64, :],
                                             w1t[cp][0:64, :],
                                             tile_position=(0, 0))
                            nc.tensor.matmul(ps1[64:128, :], xt[64:128, :],
                                             w1t[cp][64:128, :],
                                             tile_position=(64, 64))
                            off = (e * bchunk + bb) * vslab
                            nc.vector.tensor_copy(
                                vt[q][:, off:off + vslab], ps1[:, :])
                # stage 2: Y[o,c] = W^T V ; evac W0 = Y/theta into wog
                for q in range(4):
                    for o in range(O):
                        ps2 = s2psp.tile([128, nb], FP, tag="s2", name="s2")
                        for e in range(2):
                            for par in range(2):
                                r = 2 * e + par
                                # moving: vt[par*64:+64] cols (b, j) at
                                # slab e, op o
                                pa = vt[q][par * 64:(par + 1) * 64, :]
                                mov = AP(pa.tensor, pa.offset + e * bchunk
                                         * vslab + o * D,
                                         [list(pa.ap[0]), [vslab, bchunk],
                                          [1, D]])
                                nc.tensor.matmul(
                                    ps2[r * D:(r + 1) * D, :],
                                    w1t[2 * q + e][par * 64:(par + 1) * 64,
                                                   o * D:(o + 1) * D],
                                    mov,
                                    tile_position=(par * 64, r * D))
                        evac_mul(wog[o][:, q * nb:(q + 1) * nb],
                                 ps2[:, :], 1.0 / THETA)
                        t0 = accp.tile([128, ncols], HP, tag="acc",
                                       name="acc")
                        nc.scalar.mul(t0[:, q * nb:(q + 1) * nb],
                                      wog[o][:, q * nb:(q + 1) * nb],
                                      wap('W0', o))
                        nc.gpsimd.tensor_tensor(
                            Xps[:, q * nb:(q + 1) * nb],
                            t0[:, q * nb:(q + 1) * nb],
                            Xps[:, q * nb:(q + 1) * nb], op=AOP.add)

        cur = alloc_chunk()
        emit_phase_a(0, *cur)
        for ch in range(nchunk):
                Xps, wog = cur

                # ===== phase B: log iterations (op pairs interleaved) =====
                NI = 2
                for op in range(0, O, NI):
                    wcur = [wog[op + i] for i in range(NI)]
                    for j in range(NIT):
                        a, b = ITERS[j]
                        tt = []
                        for m in range(NI):
                            wt = tp.tile([128, ncols], HP, tag="t", name="t")
                            nc.vector.transpose(wt[:, :], wcur[m][:, :])
                            t = tp.tile([128, ncols], HP, tag="t", name="t")
                            nc.vector.scalar_tensor_tensor(
                                _blk(t[:, :], nblk),
                                _blk(wt[:, :], nblk),
                                float(b), _bc(aid[j], nblk),
                                op0=AOP.mult, op1=AOP.add)
                            tt.append(t)
                        gps = []
                        for m in range(NI):
                            ps = wkps.tile([128, ncols], FP, tag="wk",
                                           name="wk")
                            mmwave(ps, tt[m], wcur[m])
                            gps.append(ps)
                        gt = []
                        for m in range(NI):
                            g = gp.tile([128, ncols], HP, tag="g", name="g")
                            nc.vector.tensor_copy(g[:, :], gps[m][:, :])
                            gt.append(g)
                            acc_x(Xps[:, :], g[:, :], wap(f'G{j}', op + m))
                        kind = f'W{j + 1}' if j + 1 < NIT else 'Wf'
                        for m in range(NI):
                            ps = wkps.tile([128, ncols], FP, tag="wk",
                                           name="wk")
                            # W' = (T^T G + G^T T)/2 on SYM_SET iters:
                            # the PSUM-accumulated symmetric pair kills the
                            # asymmetric noise component that the
                            # T^T-orientation otherwise amplifies by ~a^2
                            # per iteration.
                            wnew = wogp.tile([128, ncols], HP, tag="wog",
                                             name="wog")
                            if j in SYM_SET:
                                mmwave_sym(ps, tt[m], gt[m])
                                evac_mul(wnew[:, :], ps[:, :], 0.5)
                            else:
                                mmwave(ps, tt[m], gt[m])
                                evac_copy(wnew[:, :], ps[:, :])
                            acc_x(Xps[:, :], wnew[:, :], wap(kind, op + m))
                            wcur[m] = wnew
                    # tail powers of Wf: P2 (SBUF for P3), P3 (PSUM accum)
                    p2t = []
                    for m in range(NI):
                        ps = wkps.tile([128, ncols], FP, tag="wk", name="wk")
                        mmwave(ps, wcur[m], wcur[m])
                        p2 = gp.tile([128, ncols], HP, tag="g", name="g")
                        evac_copy(p2[:, :], ps[:, :])
                        acc_x(Xps[:, :], p2[:, :], wap('P2', op + m))
                        p2t.append(p2)
                    for m in range(NI):
                        ps = wkps.tile([128, ncols], FP, tag="wk", name="wk")
                        mmwave(ps, p2t[m], wcur[m])
                        acc_x(Xps[:, :], ps[:, :], wap('P3', op + m))

                # const term: X += (COEF.one/8) * I
                nc.vector.scalar_tensor_tensor(
                    _blk(Xps[:, :], nblk), _bc(idt, nblk),
                    float(COEF['one'] / 8.0),
                    _blk(Xps[:, :], nblk), op0=AOP.mult, op1=AOP.add)

                # ===== phase C: expm =====
                xs = ctp.tile([128, ncols], HP, tag="ctmp", name="ctmp")
                evac_copy(xs[:, :], Xps[:, :])
                x2ps = wkps.tile([128, ncols], FP, tag="wk", name="wk")
                mmwave(x2ps, xs, xs)
                x2t = ctp.tile([128, ncols], HP, tag="ctmp", name="ctmp")
                evac_copy(x2t[:, :], x2ps[:, :])
                # h1 partial + plow built from xs/x2t now, so only the
                # final c6*x3t combine sits in the x3-wave bubble
                h1 = ctp.tile([128, ncols], HP, tag="ctmp", name="ctmp")
                nc.vector.scalar_tensor_tensor(
                    _blk(h1[:, :], nblk), _blk(xs[:, :], nblk),
                    float(EXPC[4]), _bc(cid3, nblk),
                    op0=AOP.mult, op1=AOP.add)
                nc.vector.scalar_tensor_tensor(
                    h1[:, :], x2t[:, :], float(EXPC[5]), h1[:, :],
                    op0=AOP.mult, op1=AOP.add)
                plow = ctp.tile([128, ncols], HP, tag="ctmp", name="ctmp")
                nc.vector.scalar_tensor_tensor(
                    _blk(plow[:, :], nblk), _blk(xs[:, :], nblk),
                    float(EXPC[1]), _bc(idt, nblk),
                    op0=AOP.mult, op1=AOP.add)
                nc.vector.scalar_tensor_tensor(
                    plow[:, :], x2t[:, :], float(EXPC[2]), plow[:, :],
                    op0=AOP.mult, op1=AOP.add)
                x3ps = wkps.tile([128, ncols], FP, tag="wk", name="wk")
                mmwave(x3ps, x2t, xs)
                x3t = ctp.tile([128, ncols], HP, tag="ctmp", name="ctmp")
                evac_copy(x3t[:, :], x3ps[:, :])
                nc.vector.scalar_tensor_tensor(
                    h1[:, :], x3t[:, :], float(EXPC[6]), h1[:, :],
                    op0=AOP.mult, op1=AOP.add)
                ppps = wkps.tile([128, ncols], FP, tag="wk", name="wk")
                mmwave(ppps, x3t, h1)
                e0 = ctp.tile([128, ncols], HP, tag="ctmp", name="ctmp")
                nc.vector.scalar_tensor_tensor(
                    e0[:, :], ppps[:, :], 1.0, plow[:, :],
                    op0=AOP.mult, op1=AOP.add)
                # overlap: next chunk's phase A fills the squaring-wave
                # dependency bubbles (in-order PE queue)
                if ch + 1 < nchunk:
                    cur = alloc_chunk()
                    emit_phase_a(ch + 1, *cur)
                e1ps = wkps.tile([128, ncols], FP, tag="wk", name="wk")
                mmwave(e1ps, e0, e0)
                e1 = ctp.tile([128, ncols], HP, tag="ctmp", name="ctmp")
                evac_copy(e1[:, :], e1ps[:, :])
                e2ps = wkps.tile([128, ncols], FP, tag="wk", name="wk")
                mmwave(e2ps, e1, e1)
                e2 = ctp.tile([128, ncols], HP, tag="ctmp", name="ctmp")
                evac_copy(e2[:, :], e2ps[:, :])
                e3ps = wkps.tile([128, ncols], FP, tag="wk", name="wk")
                mmwave(e3ps, e2, e2)
                outt = outp.tile([128, ncols], FP, tag="outt", name="outt")
                nc.vector.tensor_copy(outt[:, :], e3ps[:, :])
                # dst AP dims match src iteration order: (r,i | b,j), per q
                oa = out[:, :, :, :]
                for q in range(4):
                    dst = AP(oa.tensor,
                             ch * bchunk * C * D * D + q * 4 * D * D,
                             [[D * D, 4], [D, D],
                              [C * D * D, bchunk], [1, D]])
                    src = outt[:, q * nb:(q + 1) * nb].rearrange(
                        "p (b j) -> p b j", b=bchunk)
                    nc.sync.dma_start(dst, src)
    return nc


_NC_CACHE = {}


def prep_in_maps(x, W, weights, b_loc):
    wtab = host_wtab(np.asarray(weights))
    idt = host_idt()
    x16 = np.ascontiguousarray(x).astype(HPNP)
    W16 = np.ascontiguousarray(W).astype(HPNP)
    return [
        {"x": np.ascontiguousarray(x16[i * b_loc:(i + 1) * b_loc]),
         "W": W16, "wtab": wtab, "idt": idt}
        for i in range(NCORES)
    ]


def kernel(x: np.ndarray, W: np.ndarray, weights: np.ndarray) -> np.ndarray:
    from concourse.bass_utils import run_bass_kernel_spmd
    B = x.shape[0]
    b_loc = B // NCORES
    key = (b_loc,)
    if key not in _NC_CACHE:
        nc0 = build_nc(b_loc=b_loc, bchunk=8)
        nc0.finalize()
        _NC_CACHE[key] = nc0
    nc = _NC_CACHE[key]
    in_maps = prep_in_maps(x, W, weights, b_loc)
    res = run_bass_kernel_spmd(nc, in_maps, core_ids=list(range(NCORES)))
    return np.concatenate([r["out"] for r in res.results], axis=0)


# revision 25
# speedup vs baseline: 1.0350x; 1.0350x over previous
"""Trainium2 Bass kernel for nn_MixedOp_35562329211102.

Computes FM[b,c] = expm( sum_o weights[o] * logm( W[o,c]^T x[b,c] W[o,c] ) )
for x: [256,16,64,64] SPD, W: [6,16,64,32], weights: [6] (simplex).

Algorithm (matmul/elementwise only, no eigendecomposition):
  logm via a "W-only inverse-scaling" iteration: A = Y/theta, W0 = A,
  W_{j+1} = W_j * q_j(W_j)^2 with q_j(w) = a_j + b_j w driving the
  spectrum [1.8e-4, 0.94] into the fit window.  log(Y) is a minimax
  linear combination of the intermediates {I, W_j, G_j = W_j q_j(W_j),
  Wf^2, Wf^3} (sup err ~4.9e-3).  12 matmuls (32x32) per logm.
  expm via scaling-squaring: X = M/8, degree-6 Taylor (Paterson-
  Stockmeyer), then 3 squarings.  6 matmuls per expm.

All matmuls run in fp16 (1 PE cycle/row vs 4 for fp32) with fp32 PSUM
accumulation; X accumulates in fp32.  Since nc.tensor.matmul computes
lhsT^T @ rhs, the T operand is built from the DVE block-transposed
iterate (tt = b*W^T + a*I) so the PE computes the true T*W orientation
-- the T^T*W orientation amplifies asymmetric fp16 noise by ~a per
product and loses 1.5 digits.  End-to-end rel_l2 vs the eigh reference
~2.9e-3 (tolerance 2e-2).

Engine split: PE matmul waves; DVE: W^T transposes, T builds, G evacs;
Act: W/P2/expm evacs + scaled accumulation terms; GpSimd (Pool): the
X += term adds.  Next chunk's BiMap is emitted inside the previous
chunk's expm squarings to fill PE dependency bubbles.

Sharding: data-parallel over batch B across 8 cores (32 batches/core).
"""

import numpy as np

import concourse.bass as bass
from concourse import bacc
import concourse.mybir as mybir
from concourse.bass import AP
from concourse.tile import TileContext

FP = mybir.dt.float32
HP = mybir.dt.float16
HPNP = np.float16
ACT_EVAC = True
AOP = mybir.AluOpType

THETA = 9.0
ITERS = [
    (2.21597522, -1.63352364),
    (2.26737473, -1.64671327),
    (2.13640456, -1.45488036),
    (2.20771912, -1.45344947),
    (2.50608591, -1.40547832),
]
NIT = len(ITERS)
# With tt built from the block-transposed iterate (tt = b*W^T + a*I = T^T
# as stored), the PE's lhsT^T @ rhs computes the stable T*W orientation
# directly, so no symmetrization waves are needed.
SYM_SET = ()
# minimax fit of log(lam*theta) over lam in [1.5e-4, 0.975]; 'one' includes
# log(theta) already.
COEF = {
    'one': -7.94576706,
    'W0': 3.66999144,
    'G0': -2.02105948, 'W1': 4.17710772,
    'G1': -2.04714649, 'W2': 3.95076294,
    'G2': -2.04280673, 'W3': 3.76377375,
    'G3': -1.74073605, 'W4': 2.98926341,
    'G4': -1.04982791, 'Wf': 3.51885679,
    'P2': -1.72842878, 'P3': 0.37647398,
}
EXPC = [1.0, 1.0, 0.5, 1.0 / 6, 1.0 / 24, 1.0 / 120, 1.0 / 720]

C, O, D, DIN = 16, 6, 32, 64
NCORES = 8

WT_KINDS = ['W0'] + [f'G{j}' for j in range(NIT)] \
    + [f'W{j}' for j in range(1, NIT)] + ['Wf', 'P2', 'P3']
WT_NCOL = len(WT_KINDS) * O


def host_wtab(weights: np.ndarray) -> np.ndarray:
    """[128, WT_NCOL] per-partition scalar table: w[o]/8 * coef."""
    w8 = weights.astype(np.float64) / 8.0
    cols = []
    for k in WT_KINDS:
        s = COEF[k] / THETA if k == 'W0' else COEF[k]
        cols.append(w8 * s)
    row = np.concatenate(cols)
    return np.tile(row[None, :], (128, 1)).astype(np.float32)


def host_idt() -> np.ndarray:
    """[128, 32] fp16: 4 stacked 32x32 identities."""
    return np.tile(np.eye(D, dtype=HPNP), (4, 1))


def _bc(t, nblk):
    """broadcast a [128, D] tile AP over nblk column blocks -> [128, nblk, D]."""
    a = t[:, :]
    return AP(a.tensor, a.offset, [list(a.ap[0]), [0, nblk], [1, D]])


def _blk(ap, nblk):
    """view a [128, nblk*D] AP as [128, nblk, D]."""
    return ap.rearrange("p (n j) -> p n j", n=nblk)


def build_nc(b_loc=32, bchunk=8):
    nchunk = b_loc // bchunk
    nb = bchunk * D          # stage2 N per (o,c)
    ncols = 4 * bchunk * D   # X / wog tile width
    nblk = 4 * bchunk        # 32x32 col-blocks per wog tile
    vslab = O * D            # 192: one (e,b) slab in vt

    nc = bacc.Bacc("TRN2")
    x = nc.dram_tensor("x", [b_loc, C, DIN, DIN], HP, kind="ExternalInput")
    Wt = nc.dram_tensor("W", [O, C, DIN, D], HP, kind="ExternalInput")
    wtab_d = nc.dram_tensor("wtab", [128, WT_NCOL], FP, kind="ExternalInput")
    idt_d = nc.dram_tensor("idt", [128, D], HP, kind="ExternalInput")
    out = nc.dram_tensor("out", [b_loc, C, D, D], FP, kind="ExternalOutput")

    with TileContext(nc) as tc, (
        tc.tile_pool(name="consts", bufs=1)) as consts, (
        tc.tile_pool(name="xp", bufs=8)) as xp, (
        tc.tile_pool(name="vp", bufs=2)) as vp, (
        tc.tile_pool(name="wog", bufs=10)) as wogp, (
        tc.tile_pool(name="tp", bufs=6)) as tp, (
        tc.tile_pool(name="gp", bufs=8)) as gp, (
        tc.tile_pool(name="ct", bufs=7)) as ctp, (
        tc.tile_pool(name="outp", bufs=2)) as outp, (
        tc.tile_pool(name="accp", bufs=10)) as accp, (
        tc.tile_pool(name="xaccp", bufs=2)) as xaccp, (
        tc.tile_pool(name="s1ps", bufs=2, space="PSUM")) as s1psp, (
        tc.tile_pool(name="s2ps", bufs=2, space="PSUM")) as s2psp, (
        tc.tile_pool(name="wkps", bufs=2, space="PSUM")) as wkps:

        # ---- constants ----
        w1t = []
        for cp in range(C // 2):
            t = consts.tile([128, O * D], HP, tag=f"w1_{cp}")
            for e in range(2):
                dst = t[64 * e:64 * (e + 1), :].rearrange("p (o j) -> p o j", o=O)
                src = Wt[:, 2 * cp + e, :, :].rearrange("o p j -> p o j")
                nc.sync.dma_start(dst, src)
            w1t.append(t)
        wtab = consts.tile([128, WT_NCOL], FP, tag="wtab", name="wtab")
        nc.sync.dma_start(wtab[:, :], wtab_d[:, :])
        idt = consts.tile([128, D], HP, tag="idt", name="idt")
        nc.sync.dma_start(idt[:, :], idt_d[:, :])
        aid = []
        for j, (a, b) in enumerate(ITERS):
            t = consts.tile([128, D], HP, tag=f"aid{j}")
            nc.vector.tensor_scalar_mul(t[:, :], idt[:, :], float(a))
            aid.append(t)
        cid3 = consts.tile([128, D], HP, tag="cid3")
        nc.vector.tensor_scalar_mul(cid3[:, :], idt[:, :], float(EXPC[3]))

        def wap(kind, o):
            i = WT_KINDS.index(kind) * O + o
            return wtab[:, i:i + 1]

        def evac_copy(dst, src_):
            if ACT_EVAC:
                nc.scalar.copy(dst, src_)
            else:
                nc.vector.tensor_copy(dst, src_)

        def evac_mul(dst, src_, s):
            if ACT_EVAC:
                nc.scalar.mul(dst, src_, s)
            else:
                nc.vector.tensor_scalar_mul(dst, src_, s)

        def acc_x(xap, src_ap, wscal):
            """X += wscal * src via Act (scaled fp16 copy) + Pool (add),
            keeping DVE free for the transpose+tt chain."""
            t = accp.tile([128, ncols], HP, tag="acc", name="acc")
            nc.scalar.mul(t[:, :], src_ap, wscal)
            nc.gpsimd.tensor_tensor(xap, t[:, :], xap, op=AOP.add)

        def mmwave(dst, lhs, rhs, start=True, stop=True):
            for cb in range(nblk):
                for i in range(4):
                    sl = slice(i * D, (i + 1) * D)
                    cs = slice(cb * D, (cb + 1) * D)
                    nc.tensor.matmul(dst[sl, cs], lhs[sl, cs], rhs[sl, cs],
                                     start=start, stop=stop,
                                     tile_position=(i * D, i * D))

        # 512 fp32 cols per 2KB PSUM zero region -> 16 col-blocks
        RBLK = 16

        def mmwave_sym(dst, lhs, rhs):
            """dst = lhs^T rhs + rhs^T lhs, PSUM-accumulated.  One
            accumulation group per 2KB zero region: the first instruction
            in a region starts it, the last (second pass) stops it."""
            for ab, (l, r) in enumerate(((lhs, rhs), (rhs, lhs))):
                for cb in range(nblk):
                    for i in range(4):
                        sl = slice(i * D, (i + 1) * D)
                        cs = slice(cb * D, (cb + 1) * D)
                        nc.tensor.matmul(
                            dst[sl, cs], l[sl, cs], r[sl, cs],
                            start=(ab == 0 and cb % RBLK == 0),
                            stop=(ab == 1 and cb % RBLK == RBLK - 1),
                            skip_group_check=True,
                            tile_position=(i * D, i * D))

        def alloc_chunk():
            Xps = xaccp.tile([128, ncols], FP, tag="xacc", name="xacc")
            nc.vector.memset(Xps[:, :], 0.0)
            wog = [wogp.tile([128, ncols], HP, tag="wog", name="wog")
                   for _ in range(O)]
            return Xps, wog

        def emit_phase_a(ch, Xps, wog):
                # ===== phase A: BiMap =====
                # stage 1 (all q up front so PE isn't gated on Act evacs):
                # V[c] = x[b,c] @ W_all[c], evac to vt slabs (e, b, o, j)
                vt = []
                for q in range(4):
                    v = vp.tile([128, 2 * bchunk * vslab], HP, tag=f"v{q}",
                                name="v")
                    vt.append(v)
                for q in range(4):
                    for e in range(2):
                        cp = 2 * q + e
                        for bb in range(bchunk):
                            b = ch * bchunk + bb
                            xt = xp.tile([128, DIN], HP, tag="xt", name="xt")
                            xsrc = x[b, 2 * cp:2 * cp + 2].rearrange(
                                "c p j -> (c p) j")
                            nc.sync.dma_start(xt[:, :], xsrc)
                            ps1 = s1psp.tile([128, O * D], FP, tag="s1",
                                             name="s1")
                            nc.tensor.matmul(ps1[0:64, :], xt[0:64, :],
                                             w1t[cp][0:64, :],
                                             tile_position=(0, 0))
                            nc.tensor.matmul(ps1[64:128, :], xt[64:128, :],
                                             w1t[cp][64:128, :],
                                             tile_position=(64, 64))
                            off = (e * bchunk + bb) * vslab
                            nc.vector.tensor_copy(
                                vt[q][:, off:off + vslab], ps1[:, :])
                # stage 2: Y[o,c] = W^T V ; evac W0 = Y/theta into wog
                for q in range(4):
                    for o in range(O):
                        ps2 = s2psp.tile([128, nb], FP, tag="s2", name="s2")
                        for e in range(2):
                            for par in range(2):
                                r = 2 * e + par
                                # moving: vt[par*64:+64] cols (b, j) at
                                # slab e, op o
                                pa = vt[q][par * 64:(par + 1) * 64, :]
                                mov = AP(pa.tensor, pa.offset + e * bchunk
                                         * vslab + o * D,
                                         [list(pa.ap[0]), [vslab, bchunk],
                                          [1, D]])
                                nc.tensor.matmul(
                                    ps2[r * D:(r + 1) * D, :],
                                    w1t[2 * q + e][par * 64:(par + 1) * 64,
                                                   o * D:(o + 1) * D],
                                    mov,
                                    tile_position=(par * 64, r * D))
                        evac_mul(wog[o][:, q * nb:(q + 1) * nb],
                                 ps2[:, :], 1.0 / THETA)
                        t0 = accp.tile([128, ncols], HP, tag="acc",
                                       name="acc")
                        nc.scalar.mul(t0[:, q * nb:(q + 1) * nb],
                                      ps2[:, :], wap('W0', o))
                        nc.gpsimd.tensor_tensor(
                            Xps[:, q * nb:(q + 1) * nb],
                            t0[:, q * nb:(q + 1) * nb],
                            Xps[:, q * nb:(q + 1) * nb], op=AOP.add)

        cur = alloc_chunk()
        emit_phase_a(0, *cur)
        for ch in range(nchunk):
                Xps, wog = cur

                # ===== phase B: log iterations (op pairs interleaved) =====
                NI = 2
                for op in range(0, O, NI):
                    wcur = [wog[op + i] for i in range(NI)]
                    for j in range(NIT):
                        a, b = ITERS[j]
                        tt = []
                        for m in range(NI):
                            wt = tp.tile([128, ncols], HP, tag="t", name="t")
                            nc.vector.transpose(wt[:, :], wcur[m][:, :])
                            t = tp.tile([128, ncols], HP, tag="t", name="t")
                            nc.vector.scalar_tensor_tensor(
                                _blk(t[:, :], nblk),
                                _blk(wt[:, :], nblk),
                                float(b), _bc(aid[j], nblk),
                                op0=AOP.mult, op1=AOP.add)
                            tt.append(t)
                        gps = []
                        for m in range(NI):
                            ps = wkps.tile([128, ncols], FP, tag="wk",
                                           name="wk")
                            mmwave(ps, tt[m], wcur[m])
                            gps.append(ps)
                        gt = []
                        for m in range(NI):
                            g = gp.tile([128, ncols], HP, tag="g", name="g")
                            nc.vector.tensor_copy(g[:, :], gps[m][:, :])
                            gt.append(g)
                            acc_x(Xps[:, :], gps[m][:, :], wap(f'G{j}', op + m))
                        kind = f'W{j + 1}' if j + 1 < NIT else 'Wf'
                        for m in range(NI):
                            ps = wkps.tile([128, ncols], FP, tag="wk",
                                           name="wk")
                            # W' = (T^T G + G^T T)/2 on SYM_SET iters:
                            # the PSUM-accumulated symmetric pair kills the
                            # asymmetric noise component that the
                            # T^T-orientation otherwise amplifies by ~a^2
                            # per iteration.
                            wnew = wogp.tile([128, ncols], HP, tag="wog",
                                             name="wog")
                            if j in SYM_SET:
                                mmwave_sym(ps, tt[m], gt[m])
                                evac_mul(wnew[:, :], ps[:, :], 0.5)
                            else:
                                mmwave(ps, tt[m], gt[m])
                                evac_copy(wnew[:, :], ps[:, :])
                            acc_x(Xps[:, :], ps[:, :], wap(kind, op + m))
                            wcur[m] = wnew
                    # tail powers of Wf: P2 (SBUF for P3), P3 (PSUM accum)
                    p2t = []
                    for m in range(NI):
                        ps = wkps.tile([128, ncols], FP, tag="wk", name="wk")
                        mmwave(ps, wcur[m], wcur[m])
                        p2 = gp.tile([128, ncols], HP, tag="g", name="g")
                        evac_copy(p2[:, :], ps[:, :])
                        acc_x(Xps[:, :], ps[:, :], wap('P2', op + m))
                        p2t.append(p2)
                    for m in range(NI):
                        ps = wkps.tile([128, ncols], FP, tag="wk", name="wk")
                        mmwave(ps, p2t[m], wcur[m])
                        acc_x(Xps[:, :], ps[:, :], wap('P3', op + m))

                # const term: X += (COEF.one/8) * I
                nc.vector.scalar_tensor_tensor(
                    _blk(Xps[:, :], nblk), _bc(idt, nblk),
                    float(COEF['one'] / 8.0),
                    _blk(Xps[:, :], nblk), op0=AOP.mult, op1=AOP.add)

                # ===== phase C: expm =====
                xs = ctp.tile([128, ncols], HP, tag="ctmp", name="ctmp")
                evac_copy(xs[:, :], Xps[:, :])
                x2ps = wkps.tile([128, ncols], FP, tag="wk", name="wk")
                mmwave(x2ps, xs, xs)
                x2t = ctp.tile([128, ncols], HP, tag="ctmp", name="ctmp")
                evac_copy(x2t[:, :], x2ps[:, :])
                # h1 partial + plow built from xs/x2t now, so only the
                # final c6*x3t combine sits in the x3-wave bubble
                h1 = ctp.tile([128, ncols], HP, tag="ctmp", name="ctmp")
                nc.vector.scalar_tensor_tensor(
                    _blk(h1[:, :], nblk), _blk(xs[:, :], nblk),
                    float(EXPC[4]), _bc(cid3, nblk),
                    op0=AOP.mult, op1=AOP.add)
                nc.vector.scalar_tensor_tensor(
                    h1[:, :], x2t[:, :], float(EXPC[5]), h1[:, :],
                    op0=AOP.mult, op1=AOP.add)
                plow = ctp.tile([128, ncols], HP, tag="ctmp", name="ctmp")
                nc.vector.scalar_tensor_tensor(
                    _blk(plow[:, :], nblk), _blk(xs[:, :], nblk),
                    float(EXPC[1]), _bc(idt, nblk),
                    op0=AOP.mult, op1=AOP.add)
                nc.vector.scalar_tensor_tensor(
                    plow[:, :], x2t[:, :], float(EXPC[2]), plow[:, :],
                    op0=AOP.mult, op1=AOP.add)
                x3ps = wkps.tile([128, ncols], FP, tag="wk", name="wk")
                mmwave(x3ps, x2t, xs)
                x3t = ctp.tile([128, ncols], HP, tag="ctmp", name="ctmp")
                evac_copy(x3t[:, :], x3ps[:, :])
                nc.vector.scalar_tensor_tensor(
                    h1[:, :], x3t[:, :], float(EXPC[6]), h1[:, :],
                    op0=AOP.mult, op1=AOP.add)
                ppps = wkps.tile([128, ncols], FP, tag="wk", name="wk")
                mmwave(ppps, x3t, h1)
                e0 = ctp.tile([128, ncols], HP, tag="ctmp", name="ctmp")
                nc.vector.scalar_tensor_tensor(
                    e0[:, :], ppps[:, :], 1.0, plow[:, :],
                    op0=AOP.mult, op1=AOP.add)
                # overlap: next chunk's phase A fills the squaring-wave
                # dependency bubbles (in-order PE queue)
                if ch + 1 < nchunk:
                    cur = alloc_chunk()
                    emit_phase_a(ch + 1, *cur)
                e1ps = wkps.tile([128, ncols], FP, tag="wk", name="wk")
                mmwave(e1ps, e0, e0)
                e1 = ctp.tile([128, ncols], HP, tag="ctmp", name="ctmp")
                evac_copy(e1[:, :], e1ps[:, :])
                e2ps = wkps.tile([128, ncols], FP, tag="wk", name="wk")
                mmwave(e2ps, e1, e1)
                e2 = ctp.tile([128, ncols], HP, tag="ctmp", name="ctmp")
                evac_copy(e2[:, :], e2ps[:, :])
                e3ps = wkps.tile([128, ncols], FP, tag="wk", name="wk")
                mmwave(e3ps, e2, e2)
                outt = outp.tile([128, ncols], FP, tag="outt", name="outt")
                nc.vector.tensor_copy(outt[:, :], e3ps[:, :])
                # dst AP dims match src iteration order: (r,i | b,j), per q
                oa = out[:, :, :, :]
                for q in range(4):
                    dst = AP(oa.tensor,
                             ch * bchunk * C * D * D + q * 4 * D * D,
                             [[D * D, 4], [D, D],
                              [C * D * D, bchunk], [1, D]])
                    src = outt[:, q * nb:(q + 1) * nb].rearrange(
                        "p (b j) -> p b j", b=bchunk)
                    nc.sync.dma_start(dst, src)
    return nc


_NC_CACHE = {}


def prep_in_maps(x, W, weights, b_loc):
    wtab = host_wtab(np.asarray(weights))
    idt = host_idt()
    x16 = np.ascontiguousarray(x).astype(HPNP)
    W16 = np.ascontiguousarray(W).astype(HPNP)
    return [
        {"x": np.ascontiguousarray(x16[i * b_loc:(i + 1) * b_loc]),
         "W": W16, "wtab": wtab, "idt": idt}
        for i in range(NCORES)
    ]


def kernel(x: np.ndarray, W: np.ndarray, weights: np.ndarray) -> np.ndarray:
    from concourse.bass_utils import run_bass_kernel_spmd
    B = x.shape[0]
    b_loc = B // NCORES
    key = (b_loc,)
    if key not in _NC_CACHE:
        nc0 = build_nc(b_loc=b_loc, bchunk=8)
        nc0.finalize()
        _NC_CACHE[key] = nc0
    nc = _NC_CACHE[key]
    in_maps = prep_in_maps(x, W, weights, b_loc)
    res = run_bass_kernel_spmd(nc, in_maps, core_ids=list(range(NCORES)))
    return np.concatenate([r["out"] for r in res.results], axis=0)
